# revision 19
# baseline (speedup 1.0000x reference)
"""Adaptive embedding (4-cluster masked embedding + projection) on 8 trn2 cores.

Sharding: data-parallel over the batch dim - each of the 8 NeuronCores handles
one batch row (2048 tokens); tables replicated.

Design (v3):
- Host does routing only: cluster assignment, stable sort, shard split,
  padded int16 index arrays, parity/sub-row masks.
- All gathers are single dma_gather instructions (SWDGE ucode, 16-lane
  descriptor fan-out) instead of one indirect DMA per 128-token tile: the
  ~1us fixed SWDGE cost per instruction made per-tile gathers the bottleneck.
  dma_gather's int16 indices cap a table at 32767 rows, so:
    c0 (20000x1024): direct gather (row-major), stored as-is (no projection).
    c1 (20000x256):  transpose-mode gather -> lhsT directly, no PE transpose.
    c2 (160000x64):  table repacked as 80000x128 "super-2" rows (two adjacent
                     rows per 256B row), 3 vocab shards; transpose-mode gather
                     gives K=128 columns holding [even|odd] row pairs; a
                     per-token parity mask zeroes the wrong half and the
                     projection uses W2 stacked twice on K.
    c3 (67735x16):   table repacked 8467x128 "super-8"; one transpose-mode
                     gather, 8-way sub-row mask, W3 tiled 8x on K.
- Weights are fp8e4m3 (values ~N(0,0.64) after folding the 32x output scale):
  halves weight DMA and validated to keep max rel err ~9e-3 (gate 2e-2).
- PSUM evacuation casts fp32->fp16 and alternates DVE/Activation; output
  tensors are fp16 (bf16 for c0), upcast on host.
- The PE is kept busy with dummy matmuls during the gather phase so the HAM
  clock-gate releases (1.2 -> 2.4 GHz) before the real matmuls arrive.
"""

import os

import numpy as np
import ml_dtypes

BF16 = ml_dtypes.bfloat16
FP8 = ml_dtypes.float8_e4m3

CUTOFFS = (0, 20000, 40000, 200000, 267735)
D_PROJ = 1024
N_CORES = 8
P = 128

_BUILD_CACHE = {}
LAST_RESULT = None


def _wrap16(idx, ncols):
    """int16 index array in dma_gather's wrapped layout: item k at [k%16, k//16],
    replicated to all 8 q7 cores (16-partition groups). Returns [128, ncols]."""
    w = np.zeros((16, ncols), np.int16)
    w[:, : len(idx) // 16] = np.asarray(idx, np.int16).reshape(-1, 16).T
    return np.tile(w, (8, 1))


def _build(caps, cap2s, sb, nwarm):
    import concourse.bass as bass
    import concourse.bacc as bacc
    import concourse.tile as tile
    from concourse import mybir

    f32 = mybir.dt.float32
    bf16 = mybir.dt.bfloat16
    f16 = mybir.dt.float16
    fp8 = mybir.dt.float8e4
    i16 = mybir.dt.int16

    nt0, nt1, nt2, nt3 = caps  # 128-token tiles per cluster (nt2 = sum(cap2s)/128)
    n1, n3 = nt1 * P, nt3 * P
    n2 = sum(cap2s)
    assert nt2 * P == n2

    # uidx column sections (int16): c1, c3, c2 shards x3, c0
    u_off = np.cumsum(
        [0, nt1 * P // 16, n3 // 16] + [c // 16 for c in cap2s] + [nt0 * P // 16]
    )
    ucols = int(u_off[-1])
    mcols = n2 + n3  # masks: c2 then c3

    nc = bacc.Bacc("TRN2", target_bir_lowering=False)
    emb0 = nc.dram_tensor("emb0", [20000, 1024], bf16, kind="ExternalInput")
    emb1 = nc.dram_tensor("emb1", [20000, 256], bf16, kind="ExternalInput")
    emb2 = nc.dram_tensor("emb2", [80000, 128], bf16, kind="ExternalInput")
    emb3 = nc.dram_tensor("emb3", [8467, 128], bf16, kind="ExternalInput")
    w1d = nc.dram_tensor("w1", [256, D_PROJ], fp8, kind="ExternalInput")
    w2d = nc.dram_tensor("w2e", [128, D_PROJ], fp8, kind="ExternalInput")
    w3d = nc.dram_tensor("w3e", [128, D_PROJ], fp8, kind="ExternalInput")
    uidxd = nc.dram_tensor("uidx", [P, ucols], i16, kind="ExternalInput")
    masksd = nc.dram_tensor("masks", [P, mcols], bf16, kind="ExternalInput")
    out0 = nc.dram_tensor("out0", [nt0 * P, D_PROJ], bf16, kind="ExternalOutput")
    out1 = nc.dram_tensor("out1", [n1, D_PROJ], f16, kind="ExternalOutput")
    out2 = nc.dram_tensor("out2", [n2, D_PROJ], f16, kind="ExternalOutput")
    out3 = nc.dram_tensor("out3", [n3, D_PROJ], f16, kind="ExternalOutput")

    with tile.TileContext(nc) as tc:
        with (
            tc.tile_pool(name="const", bufs=1) as cpool,
            tc.tile_pool(name="mpsum", bufs=3, space="PSUM") as mpool,
            tc.tile_pool(name="wpsum", bufs=1, space="PSUM") as wpool,
        ):
            # --- input loads ---
            uidx = cpool.tile([P, ucols], i16, name="uidx")
            nc.sync.dma_start(out=uidx[:], in_=uidxd[:])

            w1 = [cpool.tile([P, D_PROJ], fp8, name=f"w1_{k}") for k in range(2)]
            for k in range(2):
                nc.scalar.dma_start(out=w1[k][:], in_=w1d[k * P : (k + 1) * P, :])
            masks = cpool.tile([P, mcols], bf16, name="masks")
            nc.scalar.dma_start(out=masks[:], in_=masksd[:])
            w3 = cpool.tile([P, D_PROJ], fp8, name="w3e")
            nc.scalar.dma_start(out=w3[:], in_=w3d[:])
            w2 = cpool.tile([P, D_PROJ], fp8, name="w2e")
            nc.scalar.dma_start(out=w2[:], in_=w2d[:])

            # --- dummy gather: forces the q7 "mlp" ucode library load to start
            # during the preamble instead of stalling the first real gather ---
            zidx = cpool.tile([P, 1], i16, name="zidx")
            nc.vector.memset(zidx[:], 0)
            zout = cpool.tile([P, 1, 128], bf16, name="zout")
            nc.gpsimd.dma_gather(zout[:], emb3[:], zidx[:], 16, 16, 128)

            if nwarm:
                wsrc = cpool.tile([P, 512], bf16, name="wsrc")
                nc.vector.memset(wsrc[:], 0.0)
                wps = wpool.tile([P, 512], f32, name="wps")
                for _ in range(nwarm):
                    nc.tensor.matmul(
                        wps[:], wsrc[:, 0:P], wsrc[:], start=True, stop=True
                    )

            # --- gathers (all on gpsimd SWDGE; one instruction per source) ---
            xT2 = cpool.tile([P, 1, n2], bf16, name="xT2")
            c2base = [0]
            for s in range(3):
                nc.gpsimd.dma_gather(
                    xT2[:, :, c2base[s] : c2base[s] + cap2s[s]],
                    emb2[sb[s] : sb[s + 1], :],
                    uidx[:, u_off[2 + s] : u_off[3 + s]],
                    cap2s[s], cap2s[s], 128,
                    transpose=True,
                )
                c2base.append(c2base[s] + cap2s[s])
            xT3 = cpool.tile([P, 1, n3], bf16, name="xT3")
            nc.gpsimd.dma_gather(
                xT3[:], emb3[:], uidx[:, u_off[1] : u_off[2]], n3, n3, 128,
                transpose=True,
            )
            xT1 = cpool.tile([P, 2, n1], bf16, name="xT1")  # [K%128, chunk, tok]
            nc.gpsimd.dma_gather(
                xT1[:], emb1[:], uidx[:, u_off[0] : u_off[1]], n1, n1, 256,
                transpose=True,
            )
            g0 = cpool.tile([P, nt0, 1024], bf16, name="g0")
            nc.gpsimd.dma_gather(
                g0[:], emb0[:], uidx[:, u_off[5] : u_off[6]], nt0 * P, nt0 * P,
                1024,
            )

            # --- projection pipeline ---
            xm2 = cpool.tile([P, n2], bf16, name="xm2")
            xm3 = cpool.tile([P, n3], bf16, name="xm3")
            st1 = cpool.tile([P, n1 * 8], f16, name="st1")  # n1*8 = nt1*1024
            st2 = cpool.tile([P, nt2 * D_PROJ], f16, name="st2")
            st3 = cpool.tile([P, nt3 * D_PROJ], f16, name="st3")

            ev = [0]

            def evac(dst, ps):
                e = [nc.scalar.copy, nc.vector.tensor_copy][ev[0] % 2]
                ev[0] += 1
                e(out=dst, in_=ps)



            mul = mybir.AluOpType.mult

            def project(st, t, lhs_ap, rhs_list):
                ps = mpool.tile([P, D_PROJ], f32, tag="ps", name=f"ps{ev[0]}")
                for n in range(2):
                    for k, (lap, rhs) in enumerate(rhs_list):
                        nc.tensor.matmul(
                            ps[:, n * 512 : (n + 1) * 512],
                            lap,
                            rhs[:, n * 512 : (n + 1) * 512],
                            start=(k == 0),
                            stop=(k == len(rhs_list) - 1),
                        )
                evac(st[:, t * D_PROJ : (t + 1) * D_PROJ], ps[:])

            # c2 first (its shard gathers complete first): parity mask then
            # single-chunk matmul
            for t in range(nt2):
                cols = slice(t * P, (t + 1) * P)
                nc.vector.tensor_tensor(
                    out=xm2[:, cols], in0=xT2[:, 0, cols],
                    in1=masks[:, cols], op=mul,
                )
                project(st2, t, None, [(xm2[:, cols], w2)])
            # c3: mask then single-chunk matmul
            for t in range(nt3):
                cols = slice(t * P, (t + 1) * P)
                nc.vector.tensor_tensor(
                    out=xm3[:, cols], in0=xT3[:, 0, cols],
                    in1=masks[:, n2 + t * P : n2 + (t + 1) * P], op=mul,
                )
                project(st3, t, None, [(xm3[:, cols], w3)])
            # c1: K=256 via 2 chunks, no mask
            for t in range(nt1):
                project(
                    st1, t, None,
                    [(xT1[:, k, t * P : (t + 1) * P], w1[k]) for k in range(2)],
                )

            # --- stores (few large DMAs on the sync sequencer) ---
            def store(dram, st, t0, t1):
                dst = dram[t0 * P : t1 * P, :].rearrange("(t p) i -> p t i", p=P)
                nc.sync.dma_start(out=dst, in_=st[:, t0 * D_PROJ : t1 * D_PROJ])

            q = nt2 // 3
            store(out2, st2, 0, q)
            store(out2, st2, q, 2 * q)
            store(out2, st2, 2 * q, nt2)
            store(out3, st3, 0, min(3, nt3))
            if nt3 > 3:
                store(out3, st3, 3, nt3)
            store(out1, st1, 0, nt1)
            nc.sync.dma_start(
                out=out0[:].rearrange("(t p) i -> p t i", p=P), in_=g0[:]
            )

    nc.compile()
    return nc


def kernel(tokens, emb0, emb1, emb2, emb3, proj1, proj2, proj3):
    global LAST_RESULT
    from concourse.bass_utils import run_bass_kernel_spmd

    toks = np.asarray(tokens).astype(np.int64, copy=False)
    nb, ns = toks.shape
    assert nb == N_CORES and ns % P == 0

    scale = np.float32(32.0)  # sqrt(1024): exact power of two, folded in
    emb0b = np.ascontiguousarray((np.asarray(emb0, np.float32) * scale).astype(BF16))
    emb1b = np.ascontiguousarray(np.asarray(emb1, np.float32).astype(BF16))
    emb2b = np.ascontiguousarray(
        np.asarray(emb2, np.float32).astype(BF16).reshape(80000, 128)
    )
    e3 = np.asarray(emb3, np.float32).astype(BF16)
    e3p = np.zeros((67736, 16), BF16)
    e3p[:67735] = e3
    emb3b = np.ascontiguousarray(e3p.reshape(8467, 128))
    w1 = np.ascontiguousarray((np.asarray(proj1, np.float32) * scale).astype(FP8))
    w2 = (np.asarray(proj2, np.float32) * scale).astype(FP8)
    w2e = np.ascontiguousarray(np.concatenate([w2, w2], axis=0))
    w3 = (np.asarray(proj3, np.float32) * scale).astype(FP8)
    w3e = np.ascontiguousarray(np.tile(w3, (8, 1)))

    cuts = np.asarray(CUTOFFS, dtype=np.int64)
    cluster = np.searchsorted(cuts[1:-1], toks, side="right")

    percore = []
    allsup = []
    for c in range(nb):
        cl = cluster[c]
        li = toks[c] - cuts[cl]
        d = {}
        for i in range(4):
            m = np.where(cl == i)[0]  # stable order
            d[i] = (m, li[m])
        allsup.append(d[2][1] // 2)
        percore.append(d)

    # choose c2 shard boundaries (super-row space, each shard <= 32767 rows)
    # minimizing total 128-padded capacity over the actual per-core counts
    sup_sorted = [np.sort(s) for s in allsup]
    best = None
    qs = np.arange(0.30, 0.45, 0.02)
    for q1 in qs:
        for q2 in 1.0 - qs:
            b1 = int(np.quantile(np.concatenate(allsup), q1))
            b2 = int(np.quantile(np.concatenate(allsup), q2))
            if b1 > 32767 or b2 - b1 > 32767 or 80000 - b2 > 32767:
                continue
            caps_s = []
            for lo, hi in ((0, b1), (b1, b2), (b2, 80000)):
                mx = max(
                    int(np.searchsorted(s, hi) - np.searchsorted(s, lo))
                    for s in sup_sorted
                )
                caps_s.append(-(-max(1, mx) // P) * P)
            tot = sum(caps_s)
            if best is None or tot < best[0]:
                best = (tot, (0, b1, b2, 80000), tuple(caps_s))
    _, sb, cap2s = best

    for c in range(nb):
        d = percore[c]
        m2, li2 = d[2]
        sup = li2 // 2
        par = (li2 % 2).astype(np.int8)
        shard = np.searchsorted(np.asarray(sb[1:3]), sup, side="right")
        so = np.argsort(shard, kind="stable")
        d["c2"] = (m2[so], sup[so], par[so], shard[so])

    cnt = np.array(
        [[len(percore[c][i][0]) for i in range(4)] for c in range(nb)]
    )
    caps01 = [int(-(-max(1, cnt[:, i].max()) // P)) for i in (0, 1)]
    nt3 = int(-(-max(1, cnt[:, 3].max()) // P))
    caps = (caps01[0], caps01[1], sum(cap2s) // P, nt3)
    nwarm = int(os.environ.get("KERNEL_NWARM", "0"))

    key = (caps, cap2s, sb, nwarm)
    if key not in _BUILD_CACHE:
        _BUILD_CACHE[key] = _build(caps, cap2s, sb, nwarm)
    nc = _BUILD_CACHE[key]

    nt0, nt1, nt2, _ = caps
    n1, n2, n3 = nt1 * P, sum(cap2s), nt3 * P
    in_maps = []
    for c in range(nb):
        d = percore[c]
        idx = []
        for arr, n in (
            (d[1][1], n1),
            (d[3][1] // 8, n3),
        ):
            a = np.zeros(n, np.int64)
            a[: len(arr)] = arr
            idx.append(_wrap16(a, n // 16))
        m2, sup, par, shard = d["c2"]
        for s in range(3):
            a = np.zeros(cap2s[s], np.int64)
            v = sup[shard == s] - sb[s]
            a[: len(v)] = v
            idx.append(_wrap16(a, cap2s[s] // 16))
        a = np.zeros(nt0 * P, np.int64)
        a[: len(d[0][1])] = d[0][1]
        idx.append(_wrap16(a, nt0 * P // 16))
        uidx = np.ascontiguousarray(np.concatenate(idx, axis=1))

        masks = np.zeros((P, n2 + n3), BF16)
        col = 0
        for s in range(3):
            p_s = par[shard == s]
            k = len(p_s)
            msk = np.zeros((P, cap2s[s]), np.float32)
            msk[:64, :k] = (p_s == 0).astype(np.float32)
            msk[64:, :k] = (p_s == 1).astype(np.float32)
            masks[:, col : col + cap2s[s]] = msk.astype(BF16)
            col += cap2s[s]
        sub3 = d[3][1] % 8
        msk3 = np.zeros((P, n3), np.float32)
        for b in range(8):
            sel = np.where(sub3 == b)[0]
            msk3[16 * b : 16 * (b + 1), sel] = 1.0
        masks[:, n2:] = msk3.astype(BF16)

        in_maps.append(
            {
                "emb0": emb0b, "emb1": emb1b, "emb2": emb2b, "emb3": emb3b,
                "w1": w1, "w2e": w2e, "w3e": w3e,
                "uidx": uidx, "masks": np.ascontiguousarray(masks),
            }
        )

    res = run_bass_kernel_spmd(nc, in_maps, core_ids=list(range(N_CORES)))
    LAST_RESULT = res

    out = np.empty((nb, ns, D_PROJ), np.float32)
    for c in range(nb):
        d = percore[c]
        r = res.results[c]
        out[c][d[0][0]] = np.asarray(
            r["out0"][: cnt[c, 0]], dtype=np.float32
        )
        out[c][d[1][0]] = np.asarray(
            r["out1"][: cnt[c, 1]], dtype=np.float32
        )
        m2 = d["c2"][0]
        shard = d["c2"][3]
        o2 = np.asarray(r["out2"], dtype=np.float32)
        pos = 0
        base = 0
        for s in range(3):
            k = int((shard == s).sum())
            out[c][m2[pos : pos + k]] = o2[base : base + k]
            pos += k
            base += cap2s[s]
        out[c][d[3][0]] = np.asarray(r["out3"][: cnt[c, 3]], dtype=np.float32)
    return out


# revision 20
# speedup vs baseline: 1.2052x; 1.2052x over previous
"""Adaptive embedding (4-cluster masked embedding + projection) on 8 trn2 cores.

Sharding: data-parallel over the batch dim - each of the 8 NeuronCores handles
one batch row (2048 tokens); tables replicated.

Design (v5):
- Host does routing only (cluster assignment, stable sort, padded per-tile
  int32 index columns). Device gathers rows with one built-in indirect DMA
  (SWDGE INDIRECT1D) per 128-token tile: the custom dma_gather ucode needs a
  ~12us one-time q7 library load, while INDIRECT1D is built-in and fires
  ~9us into the kernel; per-descriptor cost is ~9-11ns either way, so
  fine-grained per-tile gathers also pipeline the downstream compute.
- Tables are bf16 (emb0 pre-scaled by 32 = sqrt(D_PROJ), exact in bf16);
  gather traffic halves vs fp32.
- Projection weights are fp8e4m3 scaled by 32 (values ~N(0,0.64), well inside
  e4m3 range): validated end-to-end max rel err ~9e-3 against the 2e-2 gate.
- Per tile: PE transpose (bf16) -> PSUM -> DVE/Act evac to SBUF lhsT ->
  bf16 x fp8 matmuls (N=512 x2) -> fp32 PSUM -> evac-cast to fp16 staging.
  Transposes run one tile ahead of the matmuls so the PE never waits on the
  PSUM->SBUF round trip; evacuations alternate Scalar/Vector engines.
- Outputs are written cluster-sorted as a few large fp16 stores (bf16 for
  cluster 0, which needs no projection); host inverse-permutes and upcasts.
"""

import os

import numpy as np
import ml_dtypes

BF16 = ml_dtypes.bfloat16
FP8 = ml_dtypes.float8_e4m3

CUTOFFS = (0, 20000, 40000, 200000, 267735)
D_PROJ = 1024
DES = (1024, 256, 64, 16)
N_CORES = 8
P = 128

_BUILD_CACHE = {}
LAST_RESULT = None


def _build(caps):
    import concourse.bass as bass
    import concourse.bacc as bacc
    import concourse.tile as tile
    from concourse import mybir
    from concourse.masks import make_identity

    f32 = mybir.dt.float32
    bf16 = mybir.dt.bfloat16
    f16 = mybir.dt.float16
    fp8 = mybir.dt.float8e4
    i32 = mybir.dt.int32

    nts = list(caps)
    ntsum = sum(nts)
    # idx column layout [c0 | c1 | c2 | c3]
    col0 = [0, nts[0], nts[0] + nts[1], nts[0] + nts[1] + nts[2]]

    nc = bacc.Bacc("TRN2", target_bir_lowering=False)
    emb = [
        nc.dram_tensor(f"emb{i}", [CUTOFFS[i + 1] - CUTOFFS[i], DES[i]], bf16,
                       kind="ExternalInput")
        for i in range(4)
    ]
    proj = [None] + [
        nc.dram_tensor(f"proj{i}", [DES[i], D_PROJ], fp8, kind="ExternalInput")
        for i in (1, 2, 3)
    ]
    idx_all = nc.dram_tensor("idx_all", [P, ntsum], i32, kind="ExternalInput")
    out = [nc.dram_tensor("out0", [nts[0] * P, D_PROJ], bf16, kind="ExternalOutput")] + [
        nc.dram_tensor(f"out{i}", [nts[i] * P, D_PROJ], f16, kind="ExternalOutput")
        for i in (1, 2, 3)
    ]

    with tile.TileContext(nc) as tc:
        with (
            tc.tile_pool(name="const", bufs=1) as cpool,
            tc.tile_pool(name="xt", bufs=6) as xtpool,
            tc.tile_pool(name="tpsum", bufs=3, space="PSUM") as tppool,
            tc.tile_pool(name="mpsum", bufs=2, space="PSUM") as mpool,
        ):
            ident = cpool.tile([P, P], bf16, name="ident")
            make_identity(nc, ident)

            idxt = cpool.tile([P, ntsum], i32, name="idxt")
            nc.sync.dma_start(out=idxt[:], in_=idx_all[:])

            # weights (fp8) on the scalar engine's HWDGE; c2's first
            w2 = cpool.tile([64, D_PROJ], fp8, name="w2")
            nc.scalar.dma_start(out=w2[:], in_=proj[2][:])
            w3 = cpool.tile([16, D_PROJ], fp8, name="w3")
            nc.scalar.dma_start(out=w3[:], in_=proj[3][:])
            w1 = [cpool.tile([P, D_PROJ], fp8, name=f"w1_{k}") for k in range(2)]
            for k in range(2):
                nc.scalar.dma_start(out=w1[k][:], in_=proj[1][k * P : (k + 1) * P, :])

            g = [
                cpool.tile([P, nts[i] * DES[i]], bf16, name=f"g{i}")
                for i in range(4)
            ]

            def gather_tile(i, t):
                de = DES[i]
                nc.gpsimd.indirect_dma_start(
                    out=g[i][:, t * de : (t + 1) * de],
                    out_offset=None,
                    in_=emb[i][:],
                    in_offset=bass.IndirectOffsetOnAxis(
                        ap=idxt[:, col0[i] + t : col0[i] + t + 1], axis=0
                    ),
                )

            # gather order: pipeline the heavy cluster first; c0 (store-only
            # chain) last so the kernel tail is minimal
            gorder = (
                [(2, t) for t in range(nts[2])]
                + [(3, t) for t in range(nts[3])]
                + [(1, t) for t in range(nts[1])]
                + [(0, t) for t in range(nts[0])]
            )
            for i, t in gorder:
                gather_tile(i, t)

            st = [None] + [
                cpool.tile([P, nts[i] * D_PROJ], f16, name=f"st{i}") for i in (1, 2, 3)
            ]
            pws = {1: w1, 2: [w2], 3: [w3]}
            ev = [0]

            def evac(dst, src):
                e = [nc.scalar.copy, nc.vector.tensor_copy][ev[0] % 2]
                ev[0] += 1
                e(out=dst, in_=src)

            # software-pipelined per-tile compute: transposes run one tile
            # ahead of the matmuls so the PE never stalls on the xt evac
            compute = [(i, t) for i in (2, 3, 1) for t in range(nts[i])]

            def do_transpose(i, t):
                de = DES[i]
                nk = (de + P - 1) // P
                lhs = []
                for k in range(nk):
                    w = min(P, de - k * P)
                    tp = tppool.tile([w, P], bf16, tag="tp", name=f"tp{i}_{t}_{k}")
                    x = xtpool.tile([w, P], bf16, tag="xt", name=f"xt{i}_{t}_{k}")
                    nc.tensor.transpose(
                        out=tp[:],
                        in_=g[i][:, t * de + k * P : t * de + k * P + w],
                        identity=ident[:],
                    )
                    evac(x[:], tp[:])
                    lhs.append(x)
                return lhs

            def do_matmul(i, t, lhs):
                pw = pws[i]
                ps = mpool.tile([P, D_PROJ], f32, tag="ps", name=f"ps{i}_{t}")
                for n in range(2):
                    for k, (lap, pwk) in enumerate(zip(lhs, pw)):
                        nc.tensor.matmul(
                            ps[:, n * 512 : (n + 1) * 512],
                            lap[:],
                            pwk[:, n * 512 : (n + 1) * 512],
                            start=(k == 0),
                            stop=(k == len(lhs) - 1),
                        )
                evac(st[i][:, t * D_PROJ : (t + 1) * D_PROJ], ps[:])

            pend = None
            for i, t in compute:
                lhs = do_transpose(i, t)
                if pend is not None:
                    do_matmul(*pend)
                pend = (i, t, lhs)
            do_matmul(*pend)

            def store(i, t0, t1):
                dst = out[i][t0 * P : t1 * P, :].rearrange("(t p) i -> p t i", p=P)
                nc.sync.dma_start(out=dst, in_=st[i][:, t0 * D_PROJ : t1 * D_PROJ])

            q = max(1, nts[2] // 3)
            store(2, 0, q)
            store(2, q, 2 * q)
            store(2, 2 * q, nts[2])
            store(3, 0, min(3, nts[3]))
            if nts[3] > 3:
                store(3, 3, nts[3])
            store(1, 0, nts[1])
            nc.sync.dma_start(
                out=out[0][:].rearrange("(t p) i -> p t i", p=P),
                in_=g[0][:],
            )

    nc.compile()
    return nc


def kernel(tokens, emb0, emb1, emb2, emb3, proj1, proj2, proj3):
    global LAST_RESULT
    from concourse.bass_utils import run_bass_kernel_spmd

    toks = np.asarray(tokens).astype(np.int64, copy=False)
    nb, ns = toks.shape
    assert nb == N_CORES and ns % P == 0

    scale = np.float32(32.0)  # sqrt(1024): exact power of two, folded in
    embs_b = [
        np.ascontiguousarray((np.asarray(emb0, np.float32) * scale).astype(BF16)),
        np.ascontiguousarray(np.asarray(emb1, np.float32).astype(BF16)),
        np.ascontiguousarray(np.asarray(emb2, np.float32).astype(BF16)),
        np.ascontiguousarray(np.asarray(emb3, np.float32).astype(BF16)),
    ]
    projs_b = {
        i: np.ascontiguousarray((np.asarray(p, np.float32) * scale).astype(FP8))
        for i, p in ((1, proj1), (2, proj2), (3, proj3))
    }

    cuts = np.asarray(CUTOFFS, dtype=np.int64)
    cluster = np.searchsorted(cuts[1:-1], toks, side="right")

    orders, counts, locs = [], [], []
    for c in range(nb):
        cl = cluster[c]
        orders.append(np.argsort(cl, kind="stable"))
        counts.append(np.bincount(cl, minlength=4))
        locs.append((toks[c] - cuts[cl]).astype(np.int32))
    counts = np.stack(counts)

    caps = tuple(
        int(max(1, -(-int(counts[:, i].max()) // P))) for i in range(4)
    )
    key = caps
    if key not in _BUILD_CACHE:
        _BUILD_CACHE[key] = _build(caps)
    nc = _BUILD_CACHE[key]

    in_maps = []
    for c in range(nb):
        m = {
            "emb0": embs_b[0], "emb1": embs_b[1],
            "emb2": embs_b[2], "emb3": embs_b[3],
            "proj1": projs_b[1], "proj2": projs_b[2], "proj3": projs_b[3],
        }
        starts = np.concatenate([[0], np.cumsum(counts[c])])
        li = locs[c][orders[c]]
        cols = []
        for i in range(4):
            padded = np.zeros(caps[i] * P, np.int32)
            padded[: counts[c, i]] = li[starts[i] : starts[i + 1]]
            cols.append(padded.reshape(caps[i], P).T)
        m["idx_all"] = np.ascontiguousarray(np.concatenate(cols, axis=1))
        in_maps.append(m)

    res = run_bass_kernel_spmd(nc, in_maps, core_ids=list(range(N_CORES)))
    LAST_RESULT = res

    out = np.empty((nb, ns, D_PROJ), np.float32)
    for c in range(nb):
        segs = [
            np.asarray(res.results[c][f"out{i}"][: counts[c, i]], dtype=np.float32)
            for i in range(4)
        ]
        out[c][orders[c]] = np.concatenate(segs, axis=0)
    return out


# revision 30
# speedup vs baseline: 1.2957x; 1.0750x over previous
"""Adaptive embedding (4-cluster masked embedding + projection) on 8 trn2 cores.

Sharding: data-parallel over the batch dim - each of the 8 NeuronCores handles
one batch row (2048 tokens); tables replicated.

Design (v5):
- Host does routing only (cluster assignment, stable sort, padded per-tile
  int32 index columns). Device gathers rows with one built-in indirect DMA
  (SWDGE INDIRECT1D) per 128-token tile: the custom dma_gather ucode needs a
  ~12us one-time q7 library load, while INDIRECT1D is built-in and fires
  ~9us into the kernel; per-descriptor cost is ~9-11ns either way, so
  fine-grained per-tile gathers also pipeline the downstream compute.
- Tables are bf16 (emb0 pre-scaled by 32 = sqrt(D_PROJ), exact in bf16);
  gather traffic halves vs fp32.
- Projection weights are fp8e4m3 scaled by 32 (values ~N(0,0.64), well inside
  e4m3 range): validated end-to-end max rel err ~9e-3 against the 2e-2 gate.
- Per tile: PE transpose (bf16) -> PSUM -> DVE/Act evac to SBUF lhsT ->
  bf16 x fp8 matmuls (N=512 x2) -> fp32 PSUM -> evac-cast to fp16 staging.
  Transposes run one tile ahead of the matmuls so the PE never waits on the
  PSUM->SBUF round trip; evacuations alternate Scalar/Vector engines.
- Outputs are written cluster-sorted as a few large fp16 stores (bf16 for
  cluster 0, which needs no projection); host inverse-permutes and upcasts.
"""

import os

import numpy as np
import ml_dtypes

BF16 = ml_dtypes.bfloat16
FP8 = ml_dtypes.float8_e4m3

CUTOFFS = (0, 20000, 40000, 200000, 267735)
D_PROJ = 1024
DES = (1024, 256, 64, 16)
N_CORES = 8
P = 128

_BUILD_CACHE = {}
LAST_RESULT = None


def _build(caps):
    import concourse.bass as bass
    import concourse.bacc as bacc
    import concourse.tile as tile
    from concourse import mybir

    f32 = mybir.dt.float32
    bf16 = mybir.dt.bfloat16
    f16 = mybir.dt.float16
    fp8 = mybir.dt.float8e4
    i32 = mybir.dt.int32

    nts = list(caps)
    ntsum = sum(nts)
    # idx column layout [c0 | c1 | c2 | c3]
    col0 = [0, nts[0], nts[0] + nts[1], nts[0] + nts[1] + nts[2]]

    nc = bacc.Bacc("TRN2", target_bir_lowering=False)
    emb = [
        nc.dram_tensor(f"emb{i}", [CUTOFFS[i + 1] - CUTOFFS[i], DES[i]], bf16,
                       kind="ExternalInput")
        for i in range(4)
    ]
    identd = nc.dram_tensor("ident", [P, P], bf16, kind="ExternalInput")
    proj = [None] + [
        nc.dram_tensor(f"proj{i}", [DES[i], D_PROJ], fp8, kind="ExternalInput")
        for i in (1, 2, 3)
    ]
    idx_all = nc.dram_tensor("idx_all", [P, ntsum], i32, kind="ExternalInput")
    out = [nc.dram_tensor("out0", [nts[0] * P, D_PROJ], bf16, kind="ExternalOutput")] + [
        nc.dram_tensor(f"out{i}", [nts[i] * P, D_PROJ], f16, kind="ExternalOutput")
        for i in (1, 2, 3)
    ]

    with tile.TileContext(nc) as tc:
        with (
            tc.tile_pool(name="const", bufs=1) as cpool,
            tc.tile_pool(name="xt", bufs=6) as xtpool,
            tc.tile_pool(name="tpsum", bufs=3, space="PSUM") as tppool,
            tc.tile_pool(name="mpsum", bufs=2, space="PSUM") as mpool,
        ):
            # identity loaded from DRAM so gpsimd does nothing but gathers
            idxt = cpool.tile([P, ntsum], i32, name="idxt")
            nc.sync.dma_start(out=idxt[:], in_=idx_all[:])
            ident = cpool.tile([P, P], bf16, name="ident")
            nc.scalar.dma_start(out=ident[:], in_=identd[:])

            # weights (fp8) on the scalar engine's HWDGE; c2's first
            w2 = cpool.tile([64, D_PROJ], fp8, name="w2")
            nc.scalar.dma_start(out=w2[:], in_=proj[2][:])
            w1 = [cpool.tile([P, D_PROJ], fp8, name=f"w1_{k}") for k in range(2)]
            for k in range(2):
                nc.scalar.dma_start(out=w1[k][:], in_=proj[1][k * P : (k + 1) * P, :])
            w3 = cpool.tile([16, D_PROJ], fp8, name="w3")
            nc.scalar.dma_start(out=w3[:], in_=proj[3][:])

            # PE warmup: short junk matmuls release the HAM clock gate
            # (1.2 -> 2.4 GHz) before the first real transpose arrives
            nwarm = int(os.environ.get("KERNEL_NWARM", "30"))
            if nwarm:
                wsrc = cpool.tile([P, P], bf16, name="wsrc")
                nc.vector.memset(wsrc[:], 0.0)
                wps = tppool.tile([P, P], bf16, tag="tp", name="wps")
                for _ in range(nwarm):
                    nc.tensor.transpose(out=wps[:], in_=wsrc[:], identity=wsrc[:])

            g = [
                cpool.tile([P, nts[i] * DES[i]], bf16, name=f"g{i}")
                for i in range(4)
            ]

            def gather_tile(i, t):
                de = DES[i]
                nc.gpsimd.indirect_dma_start(
                    out=g[i][:, t * de : (t + 1) * de],
                    out_offset=None,
                    in_=emb[i][:],
                    in_offset=bass.IndirectOffsetOnAxis(
                        ap=idxt[:, col0[i] + t : col0[i] + t + 1], axis=0
                    ),
                )

            # gather order: heavy cluster 2 first; c1 (longest per-tile chain)
            # mid-stream; c3's short chains late; c0 (store-only) last so the
            # kernel tail is minimal
            h2 = nts[2] // 2
            gorder = (
                [(2, t) for t in range(h2)]
                + [(1, t) for t in range(nts[1])]
                + [(2, t) for t in range(h2, nts[2])]
                + [(3, t) for t in range(nts[3])]
                + [(0, t) for t in range(nts[0])]
            )
            for i, t in gorder:
                gather_tile(i, t)

            st = [None] + [
                cpool.tile([P, nts[i] * D_PROJ], f16, name=f"st{i}") for i in (1, 2, 3)
            ]
            pws = {1: w1, 2: [w2], 3: [w3]}
            xev = [0]

            def evac_x(dst, src):
                e = [nc.scalar.copy, nc.vector.tensor_copy][xev[0] % 2]
                xev[0] += 1
                e(out=dst, in_=src)

            oev = [0]

            def evac_out(dst, ps):
                # split each PSUM tile across both engines: balanced by
                # construction and halves the per-tile evac latency
                h = 512
                a, b = (0, h) if oev[0] % 2 else (h, 0)
                oev[0] += 1
                nc.scalar.copy(out=dst[:, a : a + h], in_=ps[:, a : a + h])
                nc.vector.tensor_copy(out=dst[:, b : b + h], in_=ps[:, b : b + h])

            # software-pipelined per-tile compute: transposes run one tile
            # ahead of the matmuls so the PE never stalls on the xt evac
            compute = (
                [(2, t) for t in range(h2)]
                + [(1, t) for t in range(nts[1])]
                + [(2, t) for t in range(h2, nts[2])]
                + [(3, t) for t in range(nts[3])]
            )

            def do_transpose(i, t):
                de = DES[i]
                nk = (de + P - 1) // P
                lhs = []
                for k in range(nk):
                    w = min(P, de - k * P)
                    tp = tppool.tile([w, P], bf16, tag="tp", name=f"tp{i}_{t}_{k}")
                    x = xtpool.tile([w, P], bf16, tag="xt", name=f"xt{i}_{t}_{k}")
                    nc.tensor.transpose(
                        out=tp[:],
                        in_=g[i][:, t * de + k * P : t * de + k * P + w],
                        identity=ident[:],
                    )
                    evac_x(x[:], tp[:])
                    lhs.append(x)
                return lhs

            def do_matmul(i, t, lhs):
                pw = pws[i]
                ps = mpool.tile([P, D_PROJ], f32, tag="ps", name=f"ps{i}_{t}")
                for n in range(2):
                    for k, (lap, pwk) in enumerate(zip(lhs, pw)):
                        nc.tensor.matmul(
                            ps[:, n * 512 : (n + 1) * 512],
                            lap[:],
                            pwk[:, n * 512 : (n + 1) * 512],
                            start=(k == 0),
                            stop=(k == len(lhs) - 1),
                        )
                evac_out(st[i][:, t * D_PROJ : (t + 1) * D_PROJ], ps[:])

            pend = None
            for i, t in compute:
                lhs = do_transpose(i, t)
                if pend is not None:
                    do_matmul(*pend)
                pend = (i, t, lhs)
            do_matmul(*pend)

            def store(i, t0, t1):
                dst = out[i][t0 * P : t1 * P, :].rearrange("(t p) i -> p t i", p=P)
                nc.sync.dma_start(out=dst, in_=st[i][:, t0 * D_PROJ : t1 * D_PROJ])

            # stores in tile-completion order, 2-3 tiles each so transfers
            # spread across the gather phase instead of bunching at the end
            def chunks(n, sz):
                return [(a, min(a + sz, n)) for a in range(0, n, sz)]

            for a, b in chunks(h2, 3):
                store(2, a, b)
            store(1, 0, nts[1])
            for a, b in [(a + h2, b + h2) for a, b in chunks(nts[2] - h2, 3)]:
                store(2, a, b)
            for a, b in chunks(nts[3], 3):
                store(3, a, b)
            nc.sync.dma_start(
                out=out[0][:].rearrange("(t p) i -> p t i", p=P),
                in_=g[0][:],
            )

    nc.compile()
    return nc


def kernel(tokens, emb0, emb1, emb2, emb3, proj1, proj2, proj3):
    global LAST_RESULT
    from concourse.bass_utils import run_bass_kernel_spmd

    toks = np.asarray(tokens).astype(np.int64, copy=False)
    nb, ns = toks.shape
    assert nb == N_CORES and ns % P == 0

    scale = np.float32(32.0)  # sqrt(1024): exact power of two, folded in
    embs_b = [
        np.ascontiguousarray((np.asarray(emb0, np.float32) * scale).astype(BF16)),
        np.ascontiguousarray(np.asarray(emb1, np.float32).astype(BF16)),
        np.ascontiguousarray(np.asarray(emb2, np.float32).astype(BF16)),
        np.ascontiguousarray(np.asarray(emb3, np.float32).astype(BF16)),
    ]
    projs_b = {
        i: np.ascontiguousarray((np.asarray(p, np.float32) * scale).astype(FP8))
        for i, p in ((1, proj1), (2, proj2), (3, proj3))
    }

    cuts = np.asarray(CUTOFFS, dtype=np.int64)
    cluster = np.searchsorted(cuts[1:-1], toks, side="right")

    orders, counts, locs = [], [], []
    for c in range(nb):
        cl = cluster[c]
        orders.append(np.argsort(cl, kind="stable"))
        counts.append(np.bincount(cl, minlength=4))
        locs.append((toks[c] - cuts[cl]).astype(np.int32))
    counts = np.stack(counts)

    caps = tuple(
        int(max(1, -(-int(counts[:, i].max()) // P))) for i in range(4)
    )
    key = caps
    if key not in _BUILD_CACHE:
        _BUILD_CACHE[key] = _build(caps)
    nc = _BUILD_CACHE[key]

    identity = np.ascontiguousarray(np.eye(P, dtype=BF16))
    in_maps = []
    for c in range(nb):
        m = {
            "emb0": embs_b[0], "emb1": embs_b[1],
            "emb2": embs_b[2], "emb3": embs_b[3],
            "proj1": projs_b[1], "proj2": projs_b[2], "proj3": projs_b[3],
            "ident": identity,
        }
        starts = np.concatenate([[0], np.cumsum(counts[c])])
        li = locs[c][orders[c]]
        cols = []
        for i in range(4):
            padded = np.zeros(caps[i] * P, np.int32)
            padded[: counts[c, i]] = li[starts[i] : starts[i + 1]]
            cols.append(padded.reshape(caps[i], P).T)
        m["idx_all"] = np.ascontiguousarray(np.concatenate(cols, axis=1))
        in_maps.append(m)

    res = run_bass_kernel_spmd(nc, in_maps, core_ids=list(range(N_CORES)))
    LAST_RESULT = res

    out = np.empty((nb, ns, D_PROJ), np.float32)
    for c in range(nb):
        segs = [
            np.asarray(res.results[c][f"out{i}"][: counts[c, i]], dtype=np.float32)
            for i in range(4)
        ]
        out[c][orders[c]] = np.concatenate(segs, axis=0)
    return out


# revision 34
# speedup vs baseline: 1.3209x; 1.0195x over previous
"""Adaptive embedding (4-cluster masked embedding + projection) on 8 trn2 cores.

Sharding: data-parallel over the batch dim - each of the 8 NeuronCores handles
one batch row (2048 tokens); tables replicated.

Design (v5):
- Host does routing only (cluster assignment, stable sort, padded per-tile
  int32 index columns). Device gathers rows with one built-in indirect DMA
  (SWDGE INDIRECT1D) per 128-token tile: the custom dma_gather ucode needs a
  ~12us one-time q7 library load, while INDIRECT1D is built-in and fires
  ~9us into the kernel; per-descriptor cost is ~9-11ns either way, so
  fine-grained per-tile gathers also pipeline the downstream compute.
- Tables are bf16 (emb0 pre-scaled by 32 = sqrt(D_PROJ), exact in bf16);
  gather traffic halves vs fp32.
- Projection weights are fp8e4m3 scaled by 32 (values ~N(0,0.64), well inside
  e4m3 range): validated end-to-end max rel err ~9e-3 against the 2e-2 gate.
- Per tile: PE transpose (bf16) -> PSUM -> DVE/Act evac to SBUF lhsT ->
  bf16 x fp8 matmuls (N=512 x2) -> fp32 PSUM -> evac-cast to fp16 staging.
  Transposes run one tile ahead of the matmuls so the PE never waits on the
  PSUM->SBUF round trip; evacuations alternate Scalar/Vector engines.
- Outputs are written cluster-sorted as a few large fp16 stores (bf16 for
  cluster 0, which needs no projection); host inverse-permutes and upcasts.
"""

import os

import numpy as np
import ml_dtypes

BF16 = ml_dtypes.bfloat16
FP8 = ml_dtypes.float8_e4m3

CUTOFFS = (0, 20000, 40000, 200000, 267735)
D_PROJ = 1024
DES = (1024, 256, 64, 16)
N_CORES = 8
P = 128

_BUILD_CACHE = {}
LAST_RESULT = None


def _build(caps):
    import concourse.bass as bass
    import concourse.bacc as bacc
    import concourse.tile as tile
    from concourse import mybir

    f32 = mybir.dt.float32
    bf16 = mybir.dt.bfloat16
    f16 = mybir.dt.float16
    fp8 = mybir.dt.float8e4
    i32 = mybir.dt.int32

    nts = list(caps)
    ntsum = sum(nts)
    # idx column layout [c0 | c1 | c2 | c3]
    col0 = [0, nts[0], nts[0] + nts[1], nts[0] + nts[1] + nts[2]]

    nc = bacc.Bacc("TRN2", target_bir_lowering=False)
    emb = [
        nc.dram_tensor(f"emb{i}", [CUTOFFS[i + 1] - CUTOFFS[i], DES[i]], bf16,
                       kind="ExternalInput")
        for i in range(4)
    ]
    identd = nc.dram_tensor("ident", [P, P], bf16, kind="ExternalInput")
    proj = [None] + [
        nc.dram_tensor(f"proj{i}", [DES[i], D_PROJ], fp8, kind="ExternalInput")
        for i in (1, 2, 3)
    ]
    idx_all = nc.dram_tensor("idx_all", [P, ntsum], i32, kind="ExternalInput")
    out = [nc.dram_tensor("out0", [nts[0] * P, D_PROJ], bf16, kind="ExternalOutput")] + [
        nc.dram_tensor(f"out{i}", [nts[i] * P, D_PROJ], f16, kind="ExternalOutput")
        for i in (1, 2, 3)
    ]

    with tile.TileContext(nc) as tc:
        with (
            tc.tile_pool(name="const", bufs=1) as cpool,
            tc.tile_pool(name="xt", bufs=6) as xtpool,
            tc.tile_pool(name="tpsum", bufs=2, space="PSUM") as tppool,
            tc.tile_pool(name="wpsum", bufs=1, space="PSUM") as wpool,
            tc.tile_pool(name="mpsum", bufs=2, space="PSUM") as mpool,
        ):
            # identity loaded from DRAM so gpsimd does nothing but gathers
            idxt = cpool.tile([P, ntsum], i32, name="idxt")
            nc.sync.dma_start(out=idxt[:], in_=idx_all[:])
            ident = cpool.tile([P, P], bf16, name="ident")
            nc.scalar.dma_start(out=ident[:], in_=identd[:])

            # weights (fp8) on the scalar engine's HWDGE; c2's first
            w2 = cpool.tile([64, D_PROJ], fp8, name="w2")
            nc.scalar.dma_start(out=w2[:], in_=proj[2][:])
            w1 = [cpool.tile([P, D_PROJ], fp8, name=f"w1_{k}") for k in range(2)]
            for k in range(2):
                nc.scalar.dma_start(out=w1[k][:], in_=proj[1][k * P : (k + 1) * P, :])
            w3 = cpool.tile([16, D_PROJ], fp8, name="w3")
            nc.scalar.dma_start(out=w3[:], in_=proj[3][:])

            # PE warmup: short junk matmuls release the HAM clock gate
            # (1.2 -> 2.4 GHz) before the first real transpose arrives, and
            # inline fillers (below) keep the activity window busy while the
            # PE is paced by the gather stream
            nwarm = int(os.environ.get("KERNEL_NWARM", "30"))
            njunk = int(os.environ.get("KERNEL_NJUNK", "3"))
            wsrc = cpool.tile([P, P], bf16, name="wsrc")
            nc.vector.memset(wsrc[:], 0.0)
            wps = wpool.tile([P, P], bf16, tag="warm", name="wps")
            for _ in range(nwarm):
                nc.tensor.transpose(out=wps[:], in_=wsrc[:], identity=wsrc[:])

            g = [
                cpool.tile([P, nts[i] * DES[i]], bf16, name=f"g{i}")
                for i in range(4)
            ]

            def gather_tile(i, t):
                de = DES[i]
                nc.gpsimd.indirect_dma_start(
                    out=g[i][:, t * de : (t + 1) * de],
                    out_offset=None,
                    in_=emb[i][:],
                    in_offset=bass.IndirectOffsetOnAxis(
                        ap=idxt[:, col0[i] + t : col0[i] + t + 1], axis=0
                    ),
                )

            # gather order: heavy cluster 2 first; c1 (longest per-tile chain)
            # mid-stream; c3's short chains late; c0 (store-only) last so the
            # kernel tail is minimal
            h2 = nts[2] // 2
            gorder = (
                [(2, t) for t in range(h2)]
                + [(1, t) for t in range(nts[1])]
                + [(2, t) for t in range(h2, nts[2])]
                + [(3, t) for t in range(nts[3])]
                + [(0, t) for t in range(nts[0])]
            )
            for i, t in gorder:
                gather_tile(i, t)

            st = [None] + [
                cpool.tile([P, nts[i] * D_PROJ], f16, name=f"st{i}") for i in (1, 2, 3)
            ]
            pws = {1: w1, 2: [w2], 3: [w3]}
            xev = [0]

            def evac_x(dst, src):
                e = [nc.scalar.copy, nc.vector.tensor_copy][xev[0] % 2]
                xev[0] += 1
                e(out=dst, in_=src)

            oev = [0]

            def evac_out(dst, ps):
                # split each PSUM tile across both engines: balanced by
                # construction and halves the per-tile evac latency
                h = 512
                a, b = (0, h) if oev[0] % 2 else (h, 0)
                oev[0] += 1
                nc.scalar.copy(out=dst[:, a : a + h], in_=ps[:, a : a + h])
                nc.vector.tensor_copy(out=dst[:, b : b + h], in_=ps[:, b : b + h])

            # software-pipelined per-tile compute: transposes run one tile
            # ahead of the matmuls so the PE never stalls on the xt evac
            compute = (
                [(2, t) for t in range(h2)]
                + [(1, t) for t in range(nts[1])]
                + [(2, t) for t in range(h2, nts[2])]
                + [(3, t) for t in range(nts[3])]
            )

            def do_transpose(i, t):
                de = DES[i]
                nk = (de + P - 1) // P
                lhs = []
                for k in range(nk):
                    w = min(P, de - k * P)
                    tp = tppool.tile([w, P], bf16, tag="tp", name=f"tp{i}_{t}_{k}")
                    x = xtpool.tile([w, P], bf16, tag="xt", name=f"xt{i}_{t}_{k}")
                    nc.tensor.transpose(
                        out=tp[:],
                        in_=g[i][:, t * de + k * P : t * de + k * P + w],
                        identity=ident[:],
                    )
                    evac_x(x[:], tp[:])
                    lhs.append(x)
                return lhs

            def do_matmul(i, t, lhs):
                pw = pws[i]
                ps = mpool.tile([P, D_PROJ], f32, tag="ps", name=f"ps{i}_{t}")
                for n in range(2):
                    for k, (lap, pwk) in enumerate(zip(lhs, pw)):
                        nc.tensor.matmul(
                            ps[:, n * 512 : (n + 1) * 512],
                            lap[:],
                            pwk[:, n * 512 : (n + 1) * 512],
                            start=(k == 0),
                            stop=(k == len(lhs) - 1),
                        )
                evac_out(st[i][:, t * D_PROJ : (t + 1) * D_PROJ], ps[:])

            pend = None
            ngp = len(compute) - 4  # tiles still paced by the gather stream
            for j, (i, t) in enumerate(compute):
                lhs = do_transpose(i, t)
                if pend is not None:
                    do_matmul(*pend)
                pend = (i, t, lhs)
                if j < ngp:
                    for _ in range(njunk):
                        nc.tensor.transpose(
                            out=wps[:], in_=wsrc[:], identity=wsrc[:]
                        )
            do_matmul(*pend)

            def store(i, t0, t1):
                dst = out[i][t0 * P : t1 * P, :].rearrange("(t p) i -> p t i", p=P)
                nc.sync.dma_start(out=dst, in_=st[i][:, t0 * D_PROJ : t1 * D_PROJ])

            # stores in tile-completion order, 2-3 tiles each so transfers
            # spread across the gather phase instead of bunching at the end
            def chunks(n, sz):
                return [(a, min(a + sz, n)) for a in range(0, n, sz)]

            for a, b in chunks(h2, 3):
                store(2, a, b)
            store(1, 0, nts[1])
            for a, b in [(a + h2, b + h2) for a, b in chunks(nts[2] - h2, 3)]:
                store(2, a, b)
            for a, b in chunks(nts[3], 3):
                store(3, a, b)
            nc.sync.dma_start(
                out=out[0][:].rearrange("(t p) i -> p t i", p=P),
                in_=g[0][:],
            )

    nc.compile()
    return nc


def kernel(tokens, emb0, emb1, emb2, emb3, proj1, proj2, proj3):
    global LAST_RESULT
    from concourse.bass_utils import run_bass_kernel_spmd

    toks = np.asarray(tokens).astype(np.int64, copy=False)
    nb, ns = toks.shape
    assert nb == N_CORES and ns % P == 0

    scale = np.float32(32.0)  # sqrt(1024): exact power of two, folded in
    embs_b = [
        np.ascontiguousarray((np.asarray(emb0, np.float32) * scale).astype(BF16)),
        np.ascontiguousarray(np.asarray(emb1, np.float32).astype(BF16)),
        np.ascontiguousarray(np.asarray(emb2, np.float32).astype(BF16)),
        np.ascontiguousarray(np.asarray(emb3, np.float32).astype(BF16)),
    ]
    projs_b = {
        i: np.ascontiguousarray((np.asarray(p, np.float32) * scale).astype(FP8))
        for i, p in ((1, proj1), (2, proj2), (3, proj3))
    }

    cuts = np.asarray(CUTOFFS, dtype=np.int64)
    cluster = np.searchsorted(cuts[1:-1], toks, side="right")

    orders, counts, locs = [], [], []
    for c in range(nb):
        cl = cluster[c]
        orders.append(np.argsort(cl, kind="stable"))
        counts.append(np.bincount(cl, minlength=4))
        locs.append((toks[c] - cuts[cl]).astype(np.int32))
    counts = np.stack(counts)

    caps = tuple(
        int(max(1, -(-int(counts[:, i].max()) // P))) for i in range(4)
    )
    key = caps
    if key not in _BUILD_CACHE:
        _BUILD_CACHE[key] = _build(caps)
    nc = _BUILD_CACHE[key]

    identity = np.ascontiguousarray(np.eye(P, dtype=BF16))
    in_maps = []
    for c in range(nb):
        m = {
            "emb0": embs_b[0], "emb1": embs_b[1],
            "emb2": embs_b[2], "emb3": embs_b[3],
            "proj1": projs_b[1], "proj2": projs_b[2], "proj3": projs_b[3],
            "ident": identity,
        }
        starts = np.concatenate([[0], np.cumsum(counts[c])])
        li = locs[c][orders[c]]
        cols = []
        for i in range(4):
            padded = np.zeros(caps[i] * P, np.int32)
            padded[: counts[c, i]] = li[starts[i] : starts[i + 1]]
            cols.append(padded.reshape(caps[i], P).T)
        m["idx_all"] = np.ascontiguousarray(np.concatenate(cols, axis=1))
        in_maps.append(m)

    res = run_bass_kernel_spmd(nc, in_maps, core_ids=list(range(N_CORES)))
    LAST_RESULT = res

    out = np.empty((nb, ns, D_PROJ), np.float32)
    for c in range(nb):
        segs = [
            np.asarray(res.results[c][f"out{i}"][: counts[c, i]], dtype=np.float32)
            for i in range(4)
        ]
        out[c][orders[c]] = np.concatenate(segs, axis=0)
    return out
